# revision 41
# baseline (speedup 1.0000x reference)
"""Deformable conv (bounded offsets) Trainium2 kernel.

Data-parallel over batch: 8 samples -> 8 NeuronCores, one sample each.

Per-core pipeline (v2 — host-prepped layouts, bf16 offset conv, gather-bound
phase 3):
  A. host builds the +/-1-padded bf16 image (two partition halves) and the
     per-chunk 2x2-quad gather tables; device loads them with single
     contiguous DMAs.
  B. offset conv: 18-ch 3x3 conv as 6 PSUM-accumulated bf16 matmul passes.
  C. clip offsets, add base coords, frac/floor via round-trick, bilinear
     weight quads (bf16) and quad-buffer gather indices (int16).
  D. per 16-row strip-chunk: one ap_gather per tap fetches all 4 bilinear
     corners (d=4 bf16) from the host-prepped quad table.
  E. gathered quads * weight quads (DVE bf16), folded 4->1, then 9
     accumulating matmuls per 512-px PSUM tile; PSUM copied out with the
     un-wrapping access pattern and DMA'd to DRAM.

Pixel order within a 2048-px strip: gather slot j -> image row
16*s + (j%16), col j//16.
"""
import sys

sys.path.insert(0, '/opt/trn_rl_repo')

import numpy as np
import ml_dtypes

B, CIN, COUT, H, W = 8, 64, 64, 128, 128
K = 9
HW = H * W
PAD1 = 130            # +/-1 padded image for offset conv
SPX = 2048            # pixels per strip (16 image rows)
NSTRIP = 8
NCHUNK = 4            # chunk c handles strips c (parts 0:64) and 4+c
BROWS = 36            # band rows per strip: py_rel in [0, 35]
BCOLS = 148           # px in [0, 147], img col = px - 9
QR = 18               # quad rows per parity copy
QC = 74
NUNIT = 4 * QR * QC   # 5328
NSUB = SPX // 512

_TAPY = np.repeat(np.arange(3), 3).astype(np.float32)
_TAPX = np.tile(np.arange(3), 3).astype(np.float32)
_CHPERM = np.concatenate([np.arange(0, 18, 2), np.arange(1, 18, 2)])

_NC_CACHE = {}


def _host_constants(weight, off_w, off_b):
    out = {}
    wk = np.ascontiguousarray(
        weight.reshape(COUT, CIN, K).transpose(1, 2, 0))       # [c, k, o]
    out["wk"] = wk.reshape(CIN, K * COUT).astype(ml_dtypes.bfloat16)

    ow = off_w[_CHPERM].reshape(18, CIN, K)                    # y-chs first
    packed = np.zeros((128, 6 * 18), np.float32)
    for p, t in enumerate([0, 3, 6]):
        packed[0:64, p * 18:(p + 1) * 18] = ow[:, :, t].T
        packed[64:128, p * 18:(p + 1) * 18] = ow[:, :, t + 1].T
    for p, t in enumerate([2, 5, 8]):
        packed[0:64, (3 + p) * 18:(4 + p) * 18] = ow[:, :, t].T
    out["offw"] = packed.astype(ml_dtypes.bfloat16)
    out["offb"] = np.ascontiguousarray(
        off_b[_CHPERM].reshape(18, 1)).astype(np.float32)

    # cby [72,16]: tap_y + 16s + r + 8 ; cbx [72,128]: tap_x + cc + 8
    cby = np.zeros((72, 16), np.float32)
    cbx = np.zeros((72, 128), np.float32)
    for s in range(NSTRIP):
        for k in range(K):
            cby[9 * s + k] = _TAPY[k] + 16 * s + np.arange(16) + 8
            cbx[9 * s + k] = _TAPX[k] + np.arange(128) + 8
    out["cby"] = cby
    out["cbx"] = cbx
    yb = np.zeros((72, 1), np.float32)
    for s in range(NSTRIP):
        yb[9 * s:9 * s + 9] = 16.0 * s
    out["ybase"] = yb
    return out


def _host_x_tables(xs):
    """Per-sample: xpad (bf16 offset-conv image) and quad gather tables."""
    x = xs.reshape(CIN, H, W).astype(np.float32)
    xpad = np.zeros((128, PAD1, PAD1), np.float32)
    xpad[0:64, 1:129, 1:129] = x
    xpad[64:128, 1:129, 0:128] = x
    xpad_bf = xpad.reshape(128, PAD1 * PAD1).astype(ml_dtypes.bfloat16)

    xb = x.astype(ml_dtypes.bfloat16).astype(np.float32)
    quads = np.zeros((NCHUNK, 128, NUNIT * 4), np.float32)
    # band per (chunk, half): rows r_lo..r_lo+35 of image, cols -9..138
    bandpad = np.zeros((CIN, H + 2 * BROWS, BCOLS), np.float32)
    bandpad[:, BROWS:BROWS + H, 9:137] = xb
    for c in range(NCHUNK):
        q5 = quads[c].reshape(128, 4, QR, QC, 4)
        for half in range(2):
            r_lo = 16 * c + 64 * half - 9
            band = bandpad[:, BROWS + r_lo: BROWS + r_lo + BROWS, :]  # [64,36,148]
            for m in range(4):
                ry, rx = m >> 1, m & 1
                for cc in range(4):
                    dy, dx = cc >> 1, cc & 1
                    nqy = QR if (ry + dy) < 2 else QR - 1
                    nqx = QC if (rx + dx) < 2 else QC - 1
                    ys = band[:, ry + dy: ry + dy + 2 * nqy: 2,
                              rx + dx: rx + dx + 2 * nqx: 2]
                    q5[64 * half:64 * half + 64, m, 0:nqy, 0:nqx, cc] = ys
    return xpad_bf, quads.astype(ml_dtypes.bfloat16)


DEBUG = False


def _build_module():
    import concourse.bass as bass
    import concourse.tile as tile
    from concourse import bacc, mybir

    f32 = mybir.dt.float32
    bf16 = mybir.dt.bfloat16
    i16 = mybir.dt.int16
    ALU = mybir.AluOpType
    ACTF = mybir.ActivationFunctionType

    nc = bacc.Bacc("TRN2", target_bir_lowering=False, debug=False,
                   enable_asserts=False, num_devices=8)

    xpad_d = nc.dram_tensor("xpad", [128, PAD1 * PAD1], bf16,
                            kind="ExternalInput").ap()
    quad_d = nc.dram_tensor("quads", [NCHUNK, 128, NUNIT * 4], bf16,
                            kind="ExternalInput").ap()
    wk_d = nc.dram_tensor("wk", [CIN, K * COUT], bf16, kind="ExternalInput").ap()
    offw_d = nc.dram_tensor("offw", [128, 108], bf16, kind="ExternalInput").ap()
    offb_d = nc.dram_tensor("offb", [18, 1], f32, kind="ExternalInput").ap()
    cby_d = nc.dram_tensor("cby", [72, 16], f32, kind="ExternalInput").ap()
    cbx_d = nc.dram_tensor("cbx", [72, 128], f32, kind="ExternalInput").ap()
    yb_d = nc.dram_tensor("ybase", [72, 1], f32, kind="ExternalInput").ap()
    out_d = nc.dram_tensor("out", [COUT, HW], f32, kind="ExternalOutput").ap()
    idxs_d = nc.dram_tensor("idxs_scratch", [72, SPX], i16,
                            kind="ExternalOutput").ap()

    with tile.TileContext(nc) as tc:
        with tc.tile_pool(name="persist", bufs=1) as persist:
            # ---- persistent tiles
            wk_t = persist.tile([128, K * COUT], bf16, tag="wk")
            nc.sync.dma_start(wk_t[0:64, :], wk_d[:])
            nc.sync.dma_start(wk_t[64:128, :], wk_d[:])
            offw_t = persist.tile([128, 108], bf16, tag="offw")
            nc.sync.dma_start(offw_t[:], offw_d[:])
            offb_t = persist.tile([18, 1], f32, tag="offb")
            nc.sync.dma_start(offb_t[:], offb_d[:])
            cby_t = persist.tile([72, 16], f32, tag="cby")
            nc.sync.dma_start(cby_t[:], cby_d[:])
            cbx_t = persist.tile([72, 128], f32, tag="cbx")
            nc.sync.dma_start(cbx_t[:], cbx_d[:])
            yb_t = persist.tile([72, 1], f32, tag="yb")
            nc.sync.dma_start(yb_t[:], yb_d[:])

            offs = persist.tile([72, 2 * SPX], f32, tag="offs")
            wq = persist.tile([72, SPX * 4], bf16, tag="wq")
            # per-chunk idx tiles: chunk-0 gathers gate on 3 DMAs, not 24
            idxwc = [persist.tile([128, K * 128], i16, tag=f"idxw{c}",
                                  name=f"idxw{c}")
                     for c in range(NCHUNK)]

            _phase1(nc, tc, tile, mybir, ALU, ACTF, f32, bf16,
                    xpad_d, offw_t, offb_t, offs)
            _phase2(nc, tc, tile, mybir, ALU, f32, bf16, i16,
                    offs, cby_t, cbx_t, yb_t, wq, idxwc, idxs_d)
            _phase3(nc, tc, tile, mybir, ALU, f32, bf16, i16,
                    quad_d, wk_t, wq, idxwc, out_d)
    nc.finalize()
    return nc


def _phase1(nc, tc, tile, mybir, ALU, ACTF, f32, bf16,
            xpad_d, offw_t, offb_t, offs):
    """Offset conv: produce offs[9s+k, 0:SPX]=y, [SPX:]=x conv outputs."""
    with tc.tile_pool(name="ph1", bufs=1) as ph1, \
         tc.tile_pool(name="ph1b", bufs=6) as ph1b, \
         tc.tile_pool(name="ph1psum", bufs=6, space="PSUM") as ph1psum:
        xpad = ph1.tile([128, PAD1 * PAD1], bf16, tag="xpad")
        nc.sync.dma_start(xpad[:], xpad_d[:])
        xp3 = xpad.rearrange("p (r c) -> p r c", r=PAD1)

        PAIRS = [(0, (0, 0)), (1, (1, 0)), (2, (2, 0))]
        SINGLES = [(3, (0, 2)), (4, (1, 2)), (5, (2, 2))]
        for s in range(NSTRIP):
            for blk in range(4):
                ps = ph1psum.tile([18, 512], f32)
                r0 = 16 * s + 4 * blk
                for j, (p, (dy, dx)) in enumerate(PAIRS):
                    nc.tensor.matmul(
                        ps[:], offw_t[:, p * 18:(p + 1) * 18],
                        xp3[:, r0 + dy:r0 + dy + 4, dx:dx + 128],
                        start=(j == 0), stop=False)
                for j, (p, (dy, dx)) in enumerate(SINGLES):
                    nc.tensor.matmul(
                        ps[:], offw_t[0:64, p * 18:(p + 1) * 18],
                        xp3[0:64, r0 + dy:r0 + dy + 4, dx:dx + 128],
                        start=False, stop=(j == 2))
                col = 512 * blk
                tmp = ph1b.tile([18, 512], f32, tag="cvout")
                nc.scalar.activation(tmp[:], ps[:], ACTF.Identity,
                                     bias=offb_t[:], scale=1.0)
                nc.sync.dma_start(offs[9 * s:9 * s + 9, col:col + 512],
                                  tmp[0:9, :])
                nc.sync.dma_start(
                    offs[9 * s:9 * s + 9, SPX + col:SPX + col + 512],
                    tmp[9:18, :])


def _phase2(nc, tc, tile, mybir, ALU, f32, bf16, i16,
            offs, cby_t, cbx_t, yb_t, wq, idxwc, idxs_d):
    """Clip+base, frac/floor, weight quads, gather indices."""
    S = SPX
    with tc.tile_pool(name="ph2", bufs=1) as ph2:
        sco = ph2.tile([72, 2 * S], f32, tag="sco")
        # clip to +/-8, add base coords (broadcast via step-0 AP dims)
        nc.vector.tensor_scalar(sco[:], offs[:], 8.0, None, ALU.min)
        cby_b = cby_t[:].broadcast_to([72, 16, 128])
        cbx_b = cbx_t[:].broadcast_to([72, 128, 16]).rearrange("p c r -> p r c")
        sco3 = sco.rearrange("p (h r c) -> p h r c", h=2, r=16)
        nc.vector.scalar_tensor_tensor(
            sco3[:, 0], sco3[:, 0], -8.0, cby_b, ALU.max, ALU.add)
        nc.vector.scalar_tensor_tensor(
            sco3[:, 1], sco3[:, 1], -8.0, cbx_b, ALU.max, ALU.add)

        # floor via round-to-nearest (+2^23) then fix-up where round > v
        ifl = ph2.tile([72, 2 * S], f32, tag="ifl")
        nc.vector.tensor_scalar(ifl[:], sco[:], 8388608.0, -8388608.0,
                                ALU.add, ALU.add)
        gtf = ph2.tile([72, 2 * S], f32, tag="gtf")
        nc.vector.tensor_tensor(gtf[:], ifl[:], sco[:], ALU.is_gt)
        nc.vector.tensor_tensor(ifl[:], ifl[:], gtf[:], ALU.subtract)
        # quad coords: q2 = (py - 16s)/2 (y), px/2 (x); fq = mod(q2,1)
        q2 = ph2.tile([72, 2 * S], f32, tag="q2")
        nc.vector.tensor_scalar(q2[:, 0:S], ifl[:, 0:S], yb_t[:], 0.5,
                                ALU.subtract, ALU.mult)
        nc.vector.tensor_scalar(q2[:, S:2 * S], ifl[:, S:2 * S], 0.5, None,
                                ALU.mult)
        qq = ph2.tile([72, 2 * S], f32, tag="qq")
        nc.vector.tensor_scalar(qq[:], q2[:], 8388608.0, -8388608.0,
                                ALU.add, ALU.add)
        fq = ph2.tile([72, 2 * S], f32, tag="fq")
        nc.vector.tensor_tensor(fq[:], qq[:], q2[:], ALU.is_gt)
        nc.vector.tensor_tensor(qq[:], qq[:], fq[:], ALU.subtract)
        nc.vector.tensor_tensor(fq[:], q2[:], qq[:], ALU.subtract)

        # idx = (2*fqy + fqx)*2*QR*QC + qy*QC + qx   (m*QR*QC part)
        idxf = ph2.tile([72, S], f32, tag="idxf")
        nc.vector.scalar_tensor_tensor(
            idxf[:], fq[:, 0:S], 2.0, fq[:, S:2 * S], ALU.mult, ALU.add)
        nc.vector.tensor_scalar(idxf[:], idxf[:], float(2 * QR * QC), None,
                                ALU.mult)
        tmpi = ph2.tile([72, S], f32, tag="tmpi")
        nc.vector.scalar_tensor_tensor(
            tmpi[:], qq[:, 0:S], float(QC), qq[:, S:2 * S], ALU.mult, ALU.add)
        nc.vector.tensor_tensor(idxf[:], idxf[:], tmpi[:], ALU.add)
        idx16 = ph2.tile([72, S], i16, tag="idx16")
        nc.vector.tensor_copy(idx16[:], idxf[:])

        # Stage idx rows in DRAM (linear memory: no partition-order limits),
        # then one wide DMA per (chunk, half) wraps all 9 taps into group 0,
        # and two doubling copies replicate to the other 3 groups.
        nc.sync.dma_start(idxs_d[:], idx16[:])
        idxd3 = idxs_d.rearrange("r (a b) -> r a b", a=16)
        for c in range(NCHUNK):
            iw = idxwc[c].rearrange("p (k e) -> p k e", k=K)
            for half in range(2):
                p0 = 64 * half
                r0 = 9 * (4 * half + c)
                nc.sync.dma_start(
                    iw[p0:p0 + 16, :, :],
                    idxd3[r0:r0 + 9, :, :].rearrange("k a b -> a k b"))
                nc.sync.dma_start(iw[p0 + 16:p0 + 32, :, :],
                                  iw[p0:p0 + 16, :, :])
                nc.sync.dma_start(iw[p0 + 32:p0 + 64, :, :],
                                  iw[p0:p0 + 32, :, :])

        # weight quads AFTER the idx pipeline: chunk-0 gathers start while
        # these run.  Slot order: slot j = q*16+pp for pixel e = pp*128+q.
        tfr = ph2.tile([72, 2 * S], f32, tag="tfr")
        nc.vector.tensor_tensor(tfr[:], sco[:], ifl[:], ALU.subtract)
        ufr = ph2.tile([72, 2 * S], f32, tag="ufr")
        nc.vector.tensor_scalar(ufr[:], tfr[:], -1.0, 1.0, ALU.mult, ALU.add)
        wq4 = wq.rearrange("p (q pp c) -> p pp q c", q=128, pp=16, c=4)
        for cc in range(4):
            dy, dx = cc >> 1, cc & 1
            vy = (tfr if dy else ufr)[:, 0:S].rearrange(
                "p (pp q) -> p pp q", pp=16)
            vx = (tfr if dx else ufr)[:, S:2 * S].rearrange(
                "p (pp q) -> p pp q", pp=16)
            nc.vector.tensor_tensor(wq4[:, :, :, cc], vy, vx, ALU.mult)


def _phase3(nc, tc, tile, mybir, ALU, f32, bf16, i16,
            quad_d, wk_t, wq, idxwc, out_d):
    """Per chunk: quad load, gather, modulate, matmul, out."""
    with tc.tile_pool(name="quad_p", bufs=2) as quad_p, \
         tc.tile_pool(name="gth_p", bufs=2) as gth_p, \
         tc.tile_pool(name="mod_p", bufs=1) as mod_p, \
         tc.tile_pool(name="f1_p", bufs=1) as f1_p, \
         tc.tile_pool(name="f2_p", bufs=2) as f2_p, \
         tc.tile_pool(name="stg_p", bufs=1) as stg_p, \
         tc.tile_pool(name="ps3", bufs=1, space="PSUM") as ps3:
        for c in range(NCHUNK):
            # ---- quad buffer [128, NUNIT, 4] bf16 (host-prepped)
            quad = quad_p.tile([128, NUNIT * 4], bf16, tag="quad")
            nc.sync.dma_start(quad[:], quad_d[c])

            # 8 persistent PSUM accumulators (4 subs x 2 halves), k-outer
            accs = [[ps3.tile([64, 512], f32, tag=f"acc{h}{s}",
                              name=f"acc{h}{s}")
                     for s in range(NSUB)] for h in range(2)]

            # ---- gather + modulate + fold + accumulate per tap
            iw = idxwc[c].rearrange("p (k e) -> p k e", k=K)
            for k in range(K):
                gth = gth_p.tile([128, SPX * 4], bf16, tag="gth")
                nc.gpsimd.ap_gather(gth[:], quad[:], iw[:, k, :],
                                    channels=128, num_elems=NUNIT, d=4,
                                    num_idxs=SPX)
                wqb = mod_p.tile([128, SPX * 4], bf16, tag="wqb")
                for half in range(2):
                    sp = 9 * (4 * half + c) + k
                    b0 = 64 * half
                    nc.sync.dma_start(wqb[b0:b0 + 1, :], wq[sp:sp + 1, :])
                    n = 1
                    while n < 64:
                        nc.sync.dma_start(wqb[b0 + n:b0 + 2 * n, :],
                                          wqb[b0:b0 + n, :])
                        n *= 2
                nc.vector.tensor_tensor(gth[:], gth[:], wqb[:], ALU.mult)
                g4 = gth.rearrange("p (e cc) -> p e cc", cc=4)
                f1 = f1_p.tile([128, SPX * 2], bf16, tag="f1")
                f12 = f1.rearrange("p (e cc) -> p e cc", cc=2)
                nc.vector.tensor_tensor(f12[:], g4[:, :, 0:2], g4[:, :, 2:4],
                                        ALU.add)
                f2 = f2_p.tile([128, SPX], bf16, tag="f2")
                nc.vector.tensor_tensor(f2[:], f12[:, :, 0], f12[:, :, 1],
                                        ALU.add)
                for half in range(2):
                    b0 = 64 * half
                    for sub in range(NSUB):
                        nc.tensor.matmul(
                            accs[half][sub][:],
                            wk_t[b0:b0 + 64, 64 * k:64 * k + 64],
                            f2[b0:b0 + 64, 512 * sub:512 * sub + 512],
                            start=(k == 0), stop=(k == 8))

            # ---- unwrapped output
            for half in range(2):
                px0 = SPX * (4 * half + c)
                stg = stg_p.tile([64, SPX], f32, tag="stg")
                for sub in range(NSUB):
                    # psum col j' -> stg[(j'%16)*128 + 32*sub + j'//16]
                    dst = stg.rearrange("p (pp q) -> p pp q", pp=16)[
                        :, :, 32 * sub:32 * sub + 32] \
                        .rearrange("p pp e -> p e pp")
                    nc.vector.tensor_copy(
                        dst, accs[half][sub][:].rearrange(
                            "p (e pp) -> p e pp", e=32))
                nc.sync.dma_start(out_d[:, px0:px0 + SPX], stg[:])


def kernel(x, weight, off_w, off_b):
    from concourse.bass_utils import run_bass_kernel_spmd

    if "nc" not in _NC_CACHE:
        _NC_CACHE["nc"] = _build_module()
    nc = _NC_CACHE["nc"]

    consts = _host_constants(np.asarray(weight, np.float32),
                             np.asarray(off_w, np.float32),
                             np.asarray(off_b, np.float32))
    xs = np.asarray(x, np.float32).reshape(B, CIN, HW)
    in_maps = []
    for i in range(B):
        xpad_bf, quads = _host_x_tables(xs[i])
        in_maps.append(dict(xpad=xpad_bf, quads=quads, **consts))
    res = run_bass_kernel_spmd(nc, in_maps, core_ids=list(range(8)))
    out = np.stack([np.asarray(res.results[i]["out"], np.float32)
                    for i in range(B)])
    return out.reshape(B, COUT, H, W)


# revision 43
# speedup vs baseline: 1.1869x; 1.1869x over previous
"""Deformable conv (bounded offsets) Trainium2 kernel.

Data-parallel over batch: 8 samples -> 8 NeuronCores, one sample each.

Per-core pipeline (v2 — host-prepped layouts, bf16 offset conv, gather-bound
phase 3):
  A. host builds the +/-1-padded bf16 image (two partition halves) and the
     per-chunk 2x2-quad gather tables; device loads them with single
     contiguous DMAs.
  B. offset conv: 18-ch 3x3 conv as 6 PSUM-accumulated bf16 matmul passes.
  C. clip offsets, add base coords, frac/floor via round-trick, bilinear
     weight quads (bf16) and quad-buffer gather indices (int16).
  D. per 16-row strip-chunk: one ap_gather per tap fetches all 4 bilinear
     corners (d=4 bf16) from the host-prepped quad table.
  E. gathered quads * weight quads (DVE bf16), folded 4->1, then 9
     accumulating matmuls per 512-px PSUM tile; PSUM copied out with the
     un-wrapping access pattern and DMA'd to DRAM.

Pixel order within a 2048-px strip: gather slot j -> image row
16*s + (j%16), col j//16.
"""
import sys

sys.path.insert(0, '/opt/trn_rl_repo')

import numpy as np
import ml_dtypes

B, CIN, COUT, H, W = 8, 64, 64, 128, 128
K = 9
HW = H * W
PAD1 = 130            # +/-1 padded image for offset conv
SPX = 2048            # pixels per strip (16 image rows)
NSTRIP = 8
NCHUNK = 4            # chunk c handles strips c (parts 0:64) and 4+c
BROWS = 36            # band rows per strip: py_rel in [0, 35]
BCOLS = 148           # px in [0, 147], img col = px - 9
QR = 18               # quad rows per parity copy
QC = 74
NUNIT = 4 * QR * QC   # 5328
NSUB = SPX // 512

_TAPY = np.repeat(np.arange(3), 3).astype(np.float32)
_TAPX = np.tile(np.arange(3), 3).astype(np.float32)
_CHPERM = np.concatenate([np.arange(0, 18, 2), np.arange(1, 18, 2)])

_NC_CACHE = {}


def _host_constants(weight, off_w, off_b):
    out = {}
    wk = np.ascontiguousarray(
        weight.reshape(COUT, CIN, K).transpose(1, 2, 0))       # [c, k, o]
    out["wk"] = wk.reshape(CIN, K * COUT).astype(ml_dtypes.bfloat16)

    ow = off_w[_CHPERM].reshape(18, CIN, K)                    # y-chs first
    packed = np.zeros((128, 6 * 18), np.float32)
    for p, t in enumerate([0, 3, 6]):
        packed[0:64, p * 18:(p + 1) * 18] = ow[:, :, t].T
        packed[64:128, p * 18:(p + 1) * 18] = ow[:, :, t + 1].T
    for p, t in enumerate([2, 5, 8]):
        packed[0:64, (3 + p) * 18:(4 + p) * 18] = ow[:, :, t].T
    out["offw"] = packed.astype(ml_dtypes.bfloat16)
    out["offb"] = np.ascontiguousarray(
        off_b[_CHPERM].reshape(18, 1)).astype(np.float32)

    # cby [72,16]: tap_y + 16s + r + 8 ; cbx [72,128]: tap_x + cc + 8
    cby = np.zeros((72, 16), np.float32)
    cbx = np.zeros((72, 128), np.float32)
    for s in range(NSTRIP):
        for k in range(K):
            cby[9 * s + k] = _TAPY[k] + 16 * s + np.arange(16) + 8
            cbx[9 * s + k] = _TAPX[k] + np.arange(128) + 8
    out["cby"] = cby
    out["cbx"] = cbx
    yb = np.zeros((72, 1), np.float32)
    for s in range(NSTRIP):
        yb[9 * s:9 * s + 9] = 16.0 * s
    out["ybase"] = yb
    return out


def _host_x_tables(xs):
    """Per-sample: xpad (bf16 offset-conv image) and quad gather tables."""
    x = xs.reshape(CIN, H, W).astype(np.float32)
    xpad = np.zeros((128, PAD1, PAD1), np.float32)
    xpad[0:64, 1:129, 1:129] = x
    xpad[64:128, 1:129, 0:128] = x
    xpad_bf = xpad.reshape(128, PAD1 * PAD1).astype(ml_dtypes.bfloat16)

    xb = x.astype(ml_dtypes.bfloat16).astype(np.float32)
    quads = np.zeros((NCHUNK, 128, NUNIT * 4), np.float32)
    # band per (chunk, half): rows r_lo..r_lo+35 of image, cols -9..138
    bandpad = np.zeros((CIN, H + 2 * BROWS, BCOLS), np.float32)
    bandpad[:, BROWS:BROWS + H, 9:137] = xb
    for c in range(NCHUNK):
        q5 = quads[c].reshape(128, 4, QR, QC, 4)
        for half in range(2):
            r_lo = 16 * c + 64 * half - 9
            band = bandpad[:, BROWS + r_lo: BROWS + r_lo + BROWS, :]  # [64,36,148]
            for m in range(4):
                ry, rx = m >> 1, m & 1
                for cc in range(4):
                    dy, dx = cc >> 1, cc & 1
                    nqy = QR if (ry + dy) < 2 else QR - 1
                    nqx = QC if (rx + dx) < 2 else QC - 1
                    ys = band[:, ry + dy: ry + dy + 2 * nqy: 2,
                              rx + dx: rx + dx + 2 * nqx: 2]
                    q5[64 * half:64 * half + 64, m, 0:nqy, 0:nqx, cc] = ys
    return xpad_bf, quads.astype(ml_dtypes.bfloat16)


DEBUG = False


def _build_module():
    import concourse.bass as bass
    import concourse.tile as tile
    from concourse import bacc, mybir

    f32 = mybir.dt.float32
    bf16 = mybir.dt.bfloat16
    i16 = mybir.dt.int16
    ALU = mybir.AluOpType
    ACTF = mybir.ActivationFunctionType

    nc = bacc.Bacc("TRN2", target_bir_lowering=False, debug=False,
                   enable_asserts=False, num_devices=8)

    xpad_d = nc.dram_tensor("xpad", [128, PAD1 * PAD1], bf16,
                            kind="ExternalInput").ap()
    quad_d = nc.dram_tensor("quads", [NCHUNK, 128, NUNIT * 4], bf16,
                            kind="ExternalInput").ap()
    wk_d = nc.dram_tensor("wk", [CIN, K * COUT], bf16, kind="ExternalInput").ap()
    offw_d = nc.dram_tensor("offw", [128, 108], bf16, kind="ExternalInput").ap()
    offb_d = nc.dram_tensor("offb", [18, 1], f32, kind="ExternalInput").ap()
    cby_d = nc.dram_tensor("cby", [72, 16], f32, kind="ExternalInput").ap()
    cbx_d = nc.dram_tensor("cbx", [72, 128], f32, kind="ExternalInput").ap()
    yb_d = nc.dram_tensor("ybase", [72, 1], f32, kind="ExternalInput").ap()
    out_d = nc.dram_tensor("out", [COUT, HW], f32, kind="ExternalOutput").ap()
    idxs_d = nc.dram_tensor("idxs_scratch", [72, SPX], i16,
                            kind="ExternalOutput").ap()

    with tile.TileContext(nc) as tc:
        with tc.tile_pool(name="persist", bufs=1) as persist:
            # ---- persistent tiles
            wk_t = persist.tile([128, K * COUT], bf16, tag="wk")
            nc.sync.dma_start(wk_t[0:64, :], wk_d[:])
            nc.sync.dma_start(wk_t[64:128, :], wk_d[:])
            offw_t = persist.tile([128, 108], bf16, tag="offw")
            nc.sync.dma_start(offw_t[:], offw_d[:])
            offb_t = persist.tile([18, 1], f32, tag="offb")
            nc.sync.dma_start(offb_t[:], offb_d[:])
            cby_t = persist.tile([72, 16], f32, tag="cby")
            nc.sync.dma_start(cby_t[:], cby_d[:])
            cbx_t = persist.tile([72, 128], f32, tag="cbx")
            nc.sync.dma_start(cbx_t[:], cbx_d[:])
            yb_t = persist.tile([72, 1], f32, tag="yb")
            nc.sync.dma_start(yb_t[:], yb_d[:])

            offs = persist.tile([72, 2 * SPX], f32, tag="offs")
            wq = persist.tile([72, SPX * 4], bf16, tag="wq")
            # per-chunk idx tiles: chunk-0 gathers gate on 3 DMAs, not 24
            idxwc = [persist.tile([128, K * 128], i16, tag=f"idxw{c}",
                                  name=f"idxw{c}")
                     for c in range(NCHUNK)]

            _phase1(nc, tc, tile, mybir, ALU, ACTF, f32, bf16,
                    xpad_d, offw_t, offb_t, offs)
            _phase2(nc, tc, tile, mybir, ALU, f32, bf16, i16,
                    offs, cby_t, cbx_t, yb_t, wq, idxwc, idxs_d)
            _phase3(nc, tc, tile, mybir, ALU, f32, bf16, i16,
                    quad_d, wk_t, wq, idxwc, out_d)
    nc.finalize()
    return nc


def _phase1(nc, tc, tile, mybir, ALU, ACTF, f32, bf16,
            xpad_d, offw_t, offb_t, offs):
    """Offset conv: produce offs[9s+k, 0:SPX]=y, [SPX:]=x conv outputs."""
    with tc.tile_pool(name="ph1", bufs=1) as ph1, \
         tc.tile_pool(name="ph1b", bufs=4) as ph1b, \
         tc.tile_pool(name="ph1psum", bufs=6, space="PSUM") as ph1psum:
        xpad = ph1.tile([128, PAD1 * PAD1], bf16, tag="xpad")
        nc.sync.dma_start(xpad[:], xpad_d[:])
        xp3 = xpad.rearrange("p (r c) -> p r c", r=PAD1)

        PAIRS = [(0, (0, 0)), (1, (1, 0)), (2, (2, 0))]
        SINGLES = [(3, (0, 2)), (4, (1, 2)), (5, (2, 2))]
        for s in range(NSTRIP):
            for blk in range(4):
                ps = ph1psum.tile([18, 512], f32)
                r0 = 16 * s + 4 * blk
                for j, (p, (dy, dx)) in enumerate(PAIRS):
                    nc.tensor.matmul(
                        ps[:], offw_t[:, p * 18:(p + 1) * 18],
                        xp3[:, r0 + dy:r0 + dy + 4, dx:dx + 128],
                        start=(j == 0), stop=False)
                for j, (p, (dy, dx)) in enumerate(SINGLES):
                    nc.tensor.matmul(
                        ps[:], offw_t[0:64, p * 18:(p + 1) * 18],
                        xp3[0:64, r0 + dy:r0 + dy + 4, dx:dx + 128],
                        start=False, stop=(j == 2))
                col = 512 * blk
                tmp = ph1b.tile([18, 512], f32, tag="cvout")
                nc.scalar.activation(tmp[:], ps[:], ACTF.Identity,
                                     bias=offb_t[:], scale=1.0)
                nc.sync.dma_start(offs[9 * s:9 * s + 9, col:col + 512],
                                  tmp[0:9, :])
                nc.sync.dma_start(
                    offs[9 * s:9 * s + 9, SPX + col:SPX + col + 512],
                    tmp[9:18, :])


def _phase2(nc, tc, tile, mybir, ALU, f32, bf16, i16,
            offs, cby_t, cbx_t, yb_t, wq, idxwc, idxs_d):
    """Clip+base, frac/floor, weight quads, gather indices."""
    S = SPX
    with tc.tile_pool(name="ph2", bufs=1) as ph2:
        sco = ph2.tile([72, 2 * S], f32, tag="sco")
        # clip to +/-8, add base coords (broadcast via step-0 AP dims)
        nc.vector.tensor_scalar(sco[:], offs[:], 8.0, None, ALU.min)
        cby_b = cby_t[:].broadcast_to([72, 16, 128])
        cbx_b = cbx_t[:].broadcast_to([72, 128, 16]).rearrange("p c r -> p r c")
        sco3 = sco.rearrange("p (h r c) -> p h r c", h=2, r=16)
        nc.vector.scalar_tensor_tensor(
            sco3[:, 0], sco3[:, 0], -8.0, cby_b, ALU.max, ALU.add)
        nc.vector.scalar_tensor_tensor(
            sco3[:, 1], sco3[:, 1], -8.0, cbx_b, ALU.max, ALU.add)

        # floor via round-to-nearest (+2^23) then fix-up where round > v
        ifl = ph2.tile([72, 2 * S], f32, tag="ifl")
        nc.vector.tensor_scalar(ifl[:], sco[:], 8388608.0, -8388608.0,
                                ALU.add, ALU.add)
        gtf = ph2.tile([72, 2 * S], f32, tag="gtf")
        nc.vector.tensor_tensor(gtf[:], ifl[:], sco[:], ALU.is_gt)
        nc.vector.tensor_tensor(ifl[:], ifl[:], gtf[:], ALU.subtract)
        # quad coords: q2 = (py - 16s)/2 (y), px/2 (x); fq = mod(q2,1)
        q2 = ph2.tile([72, 2 * S], f32, tag="q2")
        nc.vector.tensor_scalar(q2[:, 0:S], ifl[:, 0:S], yb_t[:], 0.5,
                                ALU.subtract, ALU.mult)
        nc.vector.tensor_scalar(q2[:, S:2 * S], ifl[:, S:2 * S], 0.5, None,
                                ALU.mult)
        qq = ph2.tile([72, 2 * S], f32, tag="qq")
        nc.vector.tensor_scalar(qq[:], q2[:], 8388608.0, -8388608.0,
                                ALU.add, ALU.add)
        fq = ph2.tile([72, 2 * S], f32, tag="fq")
        nc.vector.tensor_tensor(fq[:], qq[:], q2[:], ALU.is_gt)
        nc.vector.tensor_tensor(qq[:], qq[:], fq[:], ALU.subtract)
        nc.vector.tensor_tensor(fq[:], q2[:], qq[:], ALU.subtract)

        # idx = (2*fqy + fqx)*2*QR*QC + qy*QC + qx   (m*QR*QC part)
        idxf = ph2.tile([72, S], f32, tag="idxf")
        nc.vector.scalar_tensor_tensor(
            idxf[:], fq[:, 0:S], 2.0, fq[:, S:2 * S], ALU.mult, ALU.add)
        nc.vector.tensor_scalar(idxf[:], idxf[:], float(2 * QR * QC), None,
                                ALU.mult)
        tmpi = ph2.tile([72, S], f32, tag="tmpi")
        nc.vector.scalar_tensor_tensor(
            tmpi[:], qq[:, 0:S], float(QC), qq[:, S:2 * S], ALU.mult, ALU.add)
        nc.vector.tensor_tensor(idxf[:], idxf[:], tmpi[:], ALU.add)
        idx16 = ph2.tile([72, S], i16, tag="idx16")
        nc.vector.tensor_copy(idx16[:], idxf[:])

        # Stage idx rows in DRAM (linear memory: no partition-order limits),
        # then one wide DMA per (chunk, half) wraps all 9 taps into group 0,
        # and two doubling copies replicate to the other 3 groups.
        nc.sync.dma_start(idxs_d[:], idx16[:])
        idxd3 = idxs_d.rearrange("r (a b) -> r a b", a=16)
        for c in range(NCHUNK):
            iw = idxwc[c].rearrange("p (k e) -> p k e", k=K)
            for half in range(2):
                p0 = 64 * half
                r0 = 9 * (4 * half + c)
                nc.sync.dma_start(
                    iw[p0:p0 + 16, :, :],
                    idxd3[r0:r0 + 9, :, :].rearrange("k a b -> a k b"))
                nc.sync.dma_start(iw[p0 + 16:p0 + 32, :, :],
                                  iw[p0:p0 + 16, :, :])
                nc.sync.dma_start(iw[p0 + 32:p0 + 64, :, :],
                                  iw[p0:p0 + 32, :, :])

        # weight quads AFTER the idx pipeline: chunk-0 gathers start while
        # these run.  Slot order: slot j = q*16+pp for pixel e = pp*128+q.
        tfr = ph2.tile([72, 2 * S], f32, tag="tfr")
        nc.vector.tensor_tensor(tfr[:], sco[:], ifl[:], ALU.subtract)
        ufr = ph2.tile([72, 2 * S], f32, tag="ufr")
        nc.vector.tensor_scalar(ufr[:], tfr[:], -1.0, 1.0, ALU.mult, ALU.add)
        wq4 = wq.rearrange("p (q pp c) -> p pp q c", q=128, pp=16, c=4)
        for cc in range(4):
            dy, dx = cc >> 1, cc & 1
            vy = (tfr if dy else ufr)[:, 0:S].rearrange(
                "p (pp q) -> p pp q", pp=16)
            vx = (tfr if dx else ufr)[:, S:2 * S].rearrange(
                "p (pp q) -> p pp q", pp=16)
            nc.vector.tensor_tensor(wq4[:, :, :, cc], vy, vx, ALU.mult)


def _phase3(nc, tc, tile, mybir, ALU, f32, bf16, i16,
            quad_d, wk_t, wq, idxwc, out_d):
    """Per chunk: quad load, gather, modulate, matmul, out."""
    with tc.tile_pool(name="quad_p", bufs=2) as quad_p, \
         tc.tile_pool(name="gth_p", bufs=2) as gth_p, \
         tc.tile_pool(name="mod_p", bufs=1) as mod_p, \
         tc.tile_pool(name="f1_p", bufs=1) as f1_p, \
         tc.tile_pool(name="f2_p", bufs=2) as f2_p, \
         tc.tile_pool(name="stg_p", bufs=1) as stg_p, \
         tc.tile_pool(name="ps3", bufs=1, space="PSUM") as ps3:
        for c in range(NCHUNK):
            # ---- quad buffer [128, NUNIT, 4] bf16 (host-prepped)
            quad = quad_p.tile([128, NUNIT * 4], bf16, tag="quad")
            nc.sync.dma_start(quad[:], quad_d[c])

            # 8 persistent PSUM accumulators (4 subs x 2 halves), k-outer
            accs = [[ps3.tile([64, 512], f32, tag=f"acc{h}{s}",
                              name=f"acc{h}{s}")
                     for s in range(NSUB)] for h in range(2)]

            # ---- gather + modulate + fold + accumulate per tap
            iw = idxwc[c].rearrange("p (k e) -> p k e", k=K)
            for k in range(K):
                gth = gth_p.tile([128, SPX * 4], bf16, tag="gth")
                nc.gpsimd.ap_gather(gth[:], quad[:], iw[:, k, :],
                                    channels=128, num_elems=NUNIT, d=4,
                                    num_idxs=SPX)
                wqb = mod_p.tile([128, SPX * 4], bf16, tag="wqb")
                for half in range(2):
                    sp = 9 * (4 * half + c) + k
                    b0 = 64 * half
                    nc.sync.dma_start(wqb[b0:b0 + 1, :], wq[sp:sp + 1, :])
                    n = 1
                    while n < 64:
                        nc.sync.dma_start(wqb[b0 + n:b0 + 2 * n, :],
                                          wqb[b0:b0 + n, :])
                        n *= 2
                nc.vector.tensor_tensor(gth[:], gth[:], wqb[:], ALU.mult)
                g4 = gth.rearrange("p (e cc) -> p e cc", cc=4)
                f1 = f1_p.tile([128, SPX * 2], bf16, tag="f1")
                f12 = f1.rearrange("p (e cc) -> p e cc", cc=2)
                nc.vector.tensor_tensor(f12[:], g4[:, :, 0:2], g4[:, :, 2:4],
                                        ALU.add)
                f2 = f2_p.tile([128, SPX], bf16, tag="f2")
                nc.vector.tensor_tensor(f2[:], f12[:, :, 0], f12[:, :, 1],
                                        ALU.add)
                for half in range(2):
                    b0 = 64 * half
                    for sub in range(NSUB):
                        nc.tensor.matmul(
                            accs[half][sub][:],
                            wk_t[b0:b0 + 64, 64 * k:64 * k + 64],
                            f2[b0:b0 + 64, 512 * sub:512 * sub + 512],
                            start=(k == 0), stop=(k == 8))

            # ---- unwrapped output
            for half in range(2):
                px0 = SPX * (4 * half + c)
                stg = stg_p.tile([64, SPX], f32, tag="stg")
                for sub in range(NSUB):
                    # psum col j' -> stg[(j'%16)*128 + 32*sub + j'//16]
                    dst = stg.rearrange("p (pp q) -> p pp q", pp=16)[
                        :, :, 32 * sub:32 * sub + 32] \
                        .rearrange("p pp e -> p e pp")
                    nc.vector.tensor_copy(
                        dst, accs[half][sub][:].rearrange(
                            "p (e pp) -> p e pp", e=32))
                nc.sync.dma_start(out_d[:, px0:px0 + SPX], stg[:])


def kernel(x, weight, off_w, off_b):
    from concourse.bass_utils import run_bass_kernel_spmd

    if "nc" not in _NC_CACHE:
        _NC_CACHE["nc"] = _build_module()
    nc = _NC_CACHE["nc"]

    consts = _host_constants(np.asarray(weight, np.float32),
                             np.asarray(off_w, np.float32),
                             np.asarray(off_b, np.float32))
    xs = np.asarray(x, np.float32).reshape(B, CIN, HW)
    in_maps = []
    for i in range(B):
        xpad_bf, quads = _host_x_tables(xs[i])
        in_maps.append(dict(xpad=xpad_bf, quads=quads, **consts))
    res = run_bass_kernel_spmd(nc, in_maps, core_ids=list(range(8)))
    out = np.stack([np.asarray(res.results[i]["out"], np.float32)
                    for i in range(B)])
    return out.reshape(B, COUT, H, W)


# revision 44
# speedup vs baseline: 1.1898x; 1.0024x over previous
"""Deformable conv (bounded offsets) Trainium2 kernel.

Data-parallel over batch: 8 samples -> 8 NeuronCores, one sample each.

Per-core pipeline (v2 — host-prepped layouts, bf16 offset conv, gather-bound
phase 3):
  A. host builds the +/-1-padded bf16 image (two partition halves) and the
     per-chunk 2x2-quad gather tables; device loads them with single
     contiguous DMAs.
  B. offset conv: 18-ch 3x3 conv as 6 PSUM-accumulated bf16 matmul passes.
  C. clip offsets, add base coords, frac/floor via round-trick, bilinear
     weight quads (bf16) and quad-buffer gather indices (int16).
  D. per 16-row strip-chunk: one ap_gather per tap fetches all 4 bilinear
     corners (d=4 bf16) from the host-prepped quad table.
  E. gathered quads * weight quads (DVE bf16), folded 4->1, then 9
     accumulating matmuls per 512-px PSUM tile; PSUM copied out with the
     un-wrapping access pattern and DMA'd to DRAM.

Pixel order within a 2048-px strip: gather slot j -> image row
16*s + (j%16), col j//16.
"""
import sys

sys.path.insert(0, '/opt/trn_rl_repo')

import numpy as np
import ml_dtypes

B, CIN, COUT, H, W = 8, 64, 64, 128, 128
K = 9
HW = H * W
PAD1 = 130            # +/-1 padded image for offset conv
SPX = 2048            # pixels per strip (16 image rows)
NSTRIP = 8
NCHUNK = 4            # chunk c handles strips c (parts 0:64) and 4+c
BROWS = 36            # band rows per strip: py_rel in [0, 35]
BCOLS = 148           # px in [0, 147], img col = px - 9
QR = 18               # quad rows per parity copy
QC = 74
NUNIT = 4 * QR * QC   # 5328
NSUB = SPX // 512

_TAPY = np.repeat(np.arange(3), 3).astype(np.float32)
_TAPX = np.tile(np.arange(3), 3).astype(np.float32)
_CHPERM = np.concatenate([np.arange(0, 18, 2), np.arange(1, 18, 2)])

_NC_CACHE = {}


def _host_constants(weight, off_w, off_b):
    out = {}
    wk = np.ascontiguousarray(
        weight.reshape(COUT, CIN, K).transpose(1, 2, 0))       # [c, k, o]
    out["wk"] = wk.reshape(CIN, K * COUT).astype(ml_dtypes.bfloat16)

    ow = off_w[_CHPERM].reshape(18, CIN, K)                    # y-chs first
    packed = np.zeros((128, 6 * 18), np.float32)
    for p, t in enumerate([0, 3, 6]):
        packed[0:64, p * 18:(p + 1) * 18] = ow[:, :, t].T
        packed[64:128, p * 18:(p + 1) * 18] = ow[:, :, t + 1].T
    for p, t in enumerate([2, 5, 8]):
        packed[0:64, (3 + p) * 18:(4 + p) * 18] = ow[:, :, t].T
    out["offw"] = packed.astype(ml_dtypes.bfloat16)
    out["offb"] = np.ascontiguousarray(
        off_b[_CHPERM].reshape(18, 1)).astype(np.float32)

    # cby [72,16]: tap_y + 16s + r + 8 ; cbx [72,128]: tap_x + cc + 8
    cby = np.zeros((72, 16), np.float32)
    cbx = np.zeros((72, 128), np.float32)
    for s in range(NSTRIP):
        for k in range(K):
            cby[9 * s + k] = _TAPY[k] + 16 * s + np.arange(16) + 8
            cbx[9 * s + k] = _TAPX[k] + np.arange(128) + 8
    out["cby"] = cby
    out["cbx"] = cbx
    yb = np.zeros((72, 1), np.float32)
    for s in range(NSTRIP):
        yb[9 * s:9 * s + 9] = 16.0 * s
    out["ybase"] = yb
    return out


def _host_x_tables(xs):
    """Per-sample: xpad (bf16 offset-conv image) and quad gather tables."""
    x = xs.reshape(CIN, H, W).astype(np.float32)
    xpad = np.zeros((128, PAD1, PAD1), np.float32)
    xpad[0:64, 1:129, 1:129] = x
    xpad[64:128, 1:129, 0:128] = x
    xpad_bf = xpad.reshape(128, PAD1 * PAD1).astype(ml_dtypes.bfloat16)

    xb = x.astype(ml_dtypes.bfloat16).astype(np.float32)
    quads = np.zeros((NCHUNK, 128, NUNIT * 4), np.float32)
    # band per (chunk, half): rows r_lo..r_lo+35 of image, cols -9..138
    bandpad = np.zeros((CIN, H + 2 * BROWS, BCOLS), np.float32)
    bandpad[:, BROWS:BROWS + H, 9:137] = xb
    for c in range(NCHUNK):
        q5 = quads[c].reshape(128, 4, QR, QC, 4)
        for half in range(2):
            r_lo = 16 * c + 64 * half - 9
            band = bandpad[:, BROWS + r_lo: BROWS + r_lo + BROWS, :]  # [64,36,148]
            for m in range(4):
                ry, rx = m >> 1, m & 1
                for cc in range(4):
                    dy, dx = cc >> 1, cc & 1
                    nqy = QR if (ry + dy) < 2 else QR - 1
                    nqx = QC if (rx + dx) < 2 else QC - 1
                    ys = band[:, ry + dy: ry + dy + 2 * nqy: 2,
                              rx + dx: rx + dx + 2 * nqx: 2]
                    q5[64 * half:64 * half + 64, m, 0:nqy, 0:nqx, cc] = ys
    return xpad_bf, quads.astype(ml_dtypes.bfloat16)


DEBUG = False


def _build_module():
    import concourse.bass as bass
    import concourse.tile as tile
    from concourse import bacc, mybir

    f32 = mybir.dt.float32
    bf16 = mybir.dt.bfloat16
    i16 = mybir.dt.int16
    ALU = mybir.AluOpType
    ACTF = mybir.ActivationFunctionType

    nc = bacc.Bacc("TRN2", target_bir_lowering=False, debug=False,
                   enable_asserts=False, num_devices=8)

    xpad_d = nc.dram_tensor("xpad", [128, PAD1 * PAD1], bf16,
                            kind="ExternalInput").ap()
    quad_d = nc.dram_tensor("quads", [NCHUNK, 128, NUNIT * 4], bf16,
                            kind="ExternalInput").ap()
    wk_d = nc.dram_tensor("wk", [CIN, K * COUT], bf16, kind="ExternalInput").ap()
    offw_d = nc.dram_tensor("offw", [128, 108], bf16, kind="ExternalInput").ap()
    offb_d = nc.dram_tensor("offb", [18, 1], f32, kind="ExternalInput").ap()
    cby_d = nc.dram_tensor("cby", [72, 16], f32, kind="ExternalInput").ap()
    cbx_d = nc.dram_tensor("cbx", [72, 128], f32, kind="ExternalInput").ap()
    yb_d = nc.dram_tensor("ybase", [72, 1], f32, kind="ExternalInput").ap()
    out_d = nc.dram_tensor("out", [COUT, HW], f32, kind="ExternalOutput").ap()
    idxs_d = nc.dram_tensor("idxs_scratch", [72, SPX], i16,
                            kind="ExternalOutput").ap()

    with tile.TileContext(nc) as tc:
        with tc.tile_pool(name="persist", bufs=1) as persist:
            # ---- persistent tiles
            wk_t = persist.tile([128, K * COUT], bf16, tag="wk")
            nc.sync.dma_start(wk_t[0:64, :], wk_d[:])
            nc.sync.dma_start(wk_t[64:128, :], wk_d[:])
            offw_t = persist.tile([128, 108], bf16, tag="offw")
            nc.sync.dma_start(offw_t[:], offw_d[:])
            offb_t = persist.tile([18, 1], f32, tag="offb")
            nc.sync.dma_start(offb_t[:], offb_d[:])
            cby_t = persist.tile([72, 16], f32, tag="cby")
            nc.sync.dma_start(cby_t[:], cby_d[:])
            cbx_t = persist.tile([72, 128], f32, tag="cbx")
            nc.sync.dma_start(cbx_t[:], cbx_d[:])
            yb_t = persist.tile([72, 1], f32, tag="yb")
            nc.sync.dma_start(yb_t[:], yb_d[:])

            offs = persist.tile([72, 2 * SPX], f32, tag="offs")
            wq = persist.tile([72, SPX * 4], bf16, tag="wq")
            # per-chunk idx tiles: chunk-0 gathers gate on 3 DMAs, not 24
            idxwc = [persist.tile([128, K * 128], i16, tag=f"idxw{c}",
                                  name=f"idxw{c}")
                     for c in range(NCHUNK)]

            _phase1(nc, tc, tile, mybir, ALU, ACTF, f32, bf16,
                    xpad_d, offw_t, offb_t, offs)
            _phase2(nc, tc, tile, mybir, ALU, f32, bf16, i16,
                    offs, cby_t, cbx_t, yb_t, wq, idxwc, idxs_d)
            _phase3(nc, tc, tile, mybir, ALU, f32, bf16, i16,
                    quad_d, wk_t, wq, idxwc, out_d)
    nc.finalize()
    return nc


def _phase1(nc, tc, tile, mybir, ALU, ACTF, f32, bf16,
            xpad_d, offw_t, offb_t, offs):
    """Offset conv: produce offs[9s+k, 0:SPX]=y, [SPX:]=x conv outputs."""
    with tc.tile_pool(name="ph1", bufs=1) as ph1, \
         tc.tile_pool(name="ph1b", bufs=4) as ph1b, \
         tc.tile_pool(name="ph1psum", bufs=4, space="PSUM") as ph1psum:
        xpad = ph1.tile([128, PAD1 * PAD1], bf16, tag="xpad")
        nc.sync.dma_start(xpad[:], xpad_d[:])
        xp3 = xpad.rearrange("p (r c) -> p r c", r=PAD1)

        PAIRS = [(0, (0, 0)), (1, (1, 0)), (2, (2, 0))]
        SINGLES = [(3, (0, 2)), (4, (1, 2)), (5, (2, 2))]
        for s in range(NSTRIP):
            for blk in range(4):
                ps = ph1psum.tile([18, 512], f32)
                r0 = 16 * s + 4 * blk
                for j, (p, (dy, dx)) in enumerate(PAIRS):
                    nc.tensor.matmul(
                        ps[:], offw_t[:, p * 18:(p + 1) * 18],
                        xp3[:, r0 + dy:r0 + dy + 4, dx:dx + 128],
                        start=(j == 0), stop=False)
                for j, (p, (dy, dx)) in enumerate(SINGLES):
                    nc.tensor.matmul(
                        ps[:], offw_t[0:64, p * 18:(p + 1) * 18],
                        xp3[0:64, r0 + dy:r0 + dy + 4, dx:dx + 128],
                        start=False, stop=(j == 2))
                col = 512 * blk
                tmp = ph1b.tile([18, 512], f32, tag="cvout")
                nc.scalar.activation(tmp[:], ps[:], ACTF.Identity,
                                     bias=offb_t[:], scale=1.0)
                nc.sync.dma_start(offs[9 * s:9 * s + 9, col:col + 512],
                                  tmp[0:9, :])
                nc.sync.dma_start(
                    offs[9 * s:9 * s + 9, SPX + col:SPX + col + 512],
                    tmp[9:18, :])


def _phase2(nc, tc, tile, mybir, ALU, f32, bf16, i16,
            offs, cby_t, cbx_t, yb_t, wq, idxwc, idxs_d):
    """Clip+base, frac/floor, weight quads, gather indices."""
    S = SPX
    with tc.tile_pool(name="ph2", bufs=1) as ph2:
        sco = ph2.tile([72, 2 * S], f32, tag="sco")
        # clip to +/-8, add base coords (broadcast via step-0 AP dims)
        nc.vector.tensor_scalar(sco[:], offs[:], 8.0, None, ALU.min)
        cby_b = cby_t[:].broadcast_to([72, 16, 128])
        cbx_b = cbx_t[:].broadcast_to([72, 128, 16]).rearrange("p c r -> p r c")
        sco3 = sco.rearrange("p (h r c) -> p h r c", h=2, r=16)
        nc.vector.scalar_tensor_tensor(
            sco3[:, 0], sco3[:, 0], -8.0, cby_b, ALU.max, ALU.add)
        nc.vector.scalar_tensor_tensor(
            sco3[:, 1], sco3[:, 1], -8.0, cbx_b, ALU.max, ALU.add)

        # floor via round-to-nearest (+2^23) then fix-up where round > v
        ifl = ph2.tile([72, 2 * S], f32, tag="ifl")
        nc.vector.tensor_scalar(ifl[:], sco[:], 8388608.0, -8388608.0,
                                ALU.add, ALU.add)
        gtf = ph2.tile([72, 2 * S], f32, tag="gtf")
        nc.vector.tensor_tensor(gtf[:], ifl[:], sco[:], ALU.is_gt)
        nc.vector.tensor_tensor(ifl[:], ifl[:], gtf[:], ALU.subtract)
        # quad coords: q2 = (py - 16s)/2 (y), px/2 (x); fq = mod(q2,1)
        q2 = ph2.tile([72, 2 * S], f32, tag="q2")
        nc.vector.tensor_scalar(q2[:, 0:S], ifl[:, 0:S], yb_t[:], 0.5,
                                ALU.subtract, ALU.mult)
        nc.vector.tensor_scalar(q2[:, S:2 * S], ifl[:, S:2 * S], 0.5, None,
                                ALU.mult)
        qq = ph2.tile([72, 2 * S], f32, tag="qq")
        nc.vector.tensor_scalar(qq[:], q2[:], 8388608.0, -8388608.0,
                                ALU.add, ALU.add)
        fq = ph2.tile([72, 2 * S], f32, tag="fq")
        nc.vector.tensor_tensor(fq[:], qq[:], q2[:], ALU.is_gt)
        nc.vector.tensor_tensor(qq[:], qq[:], fq[:], ALU.subtract)
        nc.vector.tensor_tensor(fq[:], q2[:], qq[:], ALU.subtract)

        # idx = (2*fqy + fqx)*2*QR*QC + qy*QC + qx   (m*QR*QC part)
        idxf = ph2.tile([72, S], f32, tag="idxf")
        nc.vector.scalar_tensor_tensor(
            idxf[:], fq[:, 0:S], 2.0, fq[:, S:2 * S], ALU.mult, ALU.add)
        nc.vector.tensor_scalar(idxf[:], idxf[:], float(2 * QR * QC), None,
                                ALU.mult)
        tmpi = ph2.tile([72, S], f32, tag="tmpi")
        nc.vector.scalar_tensor_tensor(
            tmpi[:], qq[:, 0:S], float(QC), qq[:, S:2 * S], ALU.mult, ALU.add)
        nc.vector.tensor_tensor(idxf[:], idxf[:], tmpi[:], ALU.add)
        idx16 = ph2.tile([72, S], i16, tag="idx16")
        nc.vector.tensor_copy(idx16[:], idxf[:])

        # Stage idx rows in DRAM (linear memory: no partition-order limits),
        # then one wide DMA per (chunk, half) wraps all 9 taps into group 0,
        # and two doubling copies replicate to the other 3 groups.
        nc.sync.dma_start(idxs_d[:], idx16[:])
        idxd3 = idxs_d.rearrange("r (a b) -> r a b", a=16)
        for c in range(NCHUNK):
            iw = idxwc[c].rearrange("p (k e) -> p k e", k=K)
            for half in range(2):
                p0 = 64 * half
                r0 = 9 * (4 * half + c)
                nc.sync.dma_start(
                    iw[p0:p0 + 16, :, :],
                    idxd3[r0:r0 + 9, :, :].rearrange("k a b -> a k b"))
                nc.sync.dma_start(iw[p0 + 16:p0 + 32, :, :],
                                  iw[p0:p0 + 16, :, :])
                nc.sync.dma_start(iw[p0 + 32:p0 + 64, :, :],
                                  iw[p0:p0 + 32, :, :])

        # weight quads AFTER the idx pipeline: chunk-0 gathers start while
        # these run.  Slot order: slot j = q*16+pp for pixel e = pp*128+q.
        tfr = ph2.tile([72, 2 * S], f32, tag="tfr")
        nc.vector.tensor_tensor(tfr[:], sco[:], ifl[:], ALU.subtract)
        ufr = ph2.tile([72, 2 * S], f32, tag="ufr")
        nc.vector.tensor_scalar(ufr[:], tfr[:], -1.0, 1.0, ALU.mult, ALU.add)
        wq4 = wq.rearrange("p (q pp c) -> p pp q c", q=128, pp=16, c=4)
        for cc in range(4):
            dy, dx = cc >> 1, cc & 1
            vy = (tfr if dy else ufr)[:, 0:S].rearrange(
                "p (pp q) -> p pp q", pp=16)
            vx = (tfr if dx else ufr)[:, S:2 * S].rearrange(
                "p (pp q) -> p pp q", pp=16)
            nc.vector.tensor_tensor(wq4[:, :, :, cc], vy, vx, ALU.mult)


def _phase3(nc, tc, tile, mybir, ALU, f32, bf16, i16,
            quad_d, wk_t, wq, idxwc, out_d):
    """Per chunk: quad load, gather, modulate, matmul, out."""
    with tc.tile_pool(name="quad_p", bufs=2) as quad_p, \
         tc.tile_pool(name="gth_p", bufs=2) as gth_p, \
         tc.tile_pool(name="mod_p", bufs=1) as mod_p, \
         tc.tile_pool(name="f1_p", bufs=1) as f1_p, \
         tc.tile_pool(name="f2_p", bufs=2) as f2_p, \
         tc.tile_pool(name="stg_p", bufs=1) as stg_p, \
         tc.tile_pool(name="ps3", bufs=1, space="PSUM") as ps3:
        for c in range(NCHUNK):
            # ---- quad buffer [128, NUNIT, 4] bf16 (host-prepped)
            quad = quad_p.tile([128, NUNIT * 4], bf16, tag="quad")
            nc.sync.dma_start(quad[:], quad_d[c])

            # 8 persistent PSUM accumulators (4 subs x 2 halves), k-outer
            accs = [[ps3.tile([64, 512], f32, tag=f"acc{h}{s}",
                              name=f"acc{h}{s}")
                     for s in range(NSUB)] for h in range(2)]

            # ---- gather + modulate + fold + accumulate per tap
            iw = idxwc[c].rearrange("p (k e) -> p k e", k=K)
            for k in range(K):
                gth = gth_p.tile([128, SPX * 4], bf16, tag="gth")
                nc.gpsimd.ap_gather(gth[:], quad[:], iw[:, k, :],
                                    channels=128, num_elems=NUNIT, d=4,
                                    num_idxs=SPX)
                wqb = mod_p.tile([128, SPX * 4], bf16, tag="wqb")
                for half in range(2):
                    sp = 9 * (4 * half + c) + k
                    b0 = 64 * half
                    nc.sync.dma_start(wqb[b0:b0 + 1, :], wq[sp:sp + 1, :])
                    n = 1
                    while n < 64:
                        nc.sync.dma_start(wqb[b0 + n:b0 + 2 * n, :],
                                          wqb[b0:b0 + n, :])
                        n *= 2
                nc.vector.tensor_tensor(gth[:], gth[:], wqb[:], ALU.mult)
                g4 = gth.rearrange("p (e cc) -> p e cc", cc=4)
                f1 = f1_p.tile([128, SPX * 2], bf16, tag="f1")
                f12 = f1.rearrange("p (e cc) -> p e cc", cc=2)
                nc.vector.tensor_tensor(f12[:], g4[:, :, 0:2], g4[:, :, 2:4],
                                        ALU.add)
                f2 = f2_p.tile([128, SPX], bf16, tag="f2")
                nc.vector.tensor_tensor(f2[:], f12[:, :, 0], f12[:, :, 1],
                                        ALU.add)
                for half in range(2):
                    b0 = 64 * half
                    for sub in range(NSUB):
                        nc.tensor.matmul(
                            accs[half][sub][:],
                            wk_t[b0:b0 + 64, 64 * k:64 * k + 64],
                            f2[b0:b0 + 64, 512 * sub:512 * sub + 512],
                            start=(k == 0), stop=(k == 8))

            # ---- unwrapped output
            for half in range(2):
                px0 = SPX * (4 * half + c)
                stg = stg_p.tile([64, SPX], f32, tag="stg")
                for sub in range(NSUB):
                    # psum col j' -> stg[(j'%16)*128 + 32*sub + j'//16]
                    dst = stg.rearrange("p (pp q) -> p pp q", pp=16)[
                        :, :, 32 * sub:32 * sub + 32] \
                        .rearrange("p pp e -> p e pp")
                    nc.vector.tensor_copy(
                        dst, accs[half][sub][:].rearrange(
                            "p (e pp) -> p e pp", e=32))
                nc.sync.dma_start(out_d[:, px0:px0 + SPX], stg[:])


def kernel(x, weight, off_w, off_b):
    from concourse.bass_utils import run_bass_kernel_spmd

    if "nc" not in _NC_CACHE:
        _NC_CACHE["nc"] = _build_module()
    nc = _NC_CACHE["nc"]

    consts = _host_constants(np.asarray(weight, np.float32),
                             np.asarray(off_w, np.float32),
                             np.asarray(off_b, np.float32))
    xs = np.asarray(x, np.float32).reshape(B, CIN, HW)
    in_maps = []
    for i in range(B):
        xpad_bf, quads = _host_x_tables(xs[i])
        in_maps.append(dict(xpad=xpad_bf, quads=quads, **consts))
    res = run_bass_kernel_spmd(nc, in_maps, core_ids=list(range(8)))
    out = np.stack([np.asarray(res.results[i]["out"], np.float32)
                    for i in range(B)])
    return out.reshape(B, COUT, H, W)
